# revision 18
# baseline (speedup 1.0000x reference)
"""Multi-head attention (RoPE + causal softmax) Trainium2 Bass kernel.

Problem: nn_MultiHeadAttention (B=16, S=512, D=1024, H=16, Hd=64).
Sharding: data-parallel over batch - 2 batches per core on 8 NeuronCores.

v3. Key structure (per core, T=1024 tokens feature-major):
  * Inputs stream in first-use order on the Sync HWDGE queue (+ Scalar for
    the early critical tiles, gpsimd for consts) so the PE starts ~2us in.
  * Softmax normalize: ACT copies av psum->sbuf bf16 (frees PSUM in
    ~1.8us), sums row [1,512] -> [128,4] via Sync HWDGE sbuf-sbuf DMA,
    DVE reciprocal, DMA back, gpsimd partition-broadcast, bf16 2x mul.
  * RoPE: pre (ACT copy), rpre (DVE cast), then 3 bf16 2x DVE TTs.
  * Causal mask muls batched over both heads via stride-0 broadcast AP.
  * bf16 output, DVE (phase D) / ACT (phase E) psum->bf16 copies,
    out DMA on gpsimd queue.
  * Schedule: A: all b0 q/k groups; B: v(b0,nh0); C: attn(b0) pairs with
    qk(b1)/v fillers; D: attn(b1) pairs with qk-tail/v/wo(b0) fillers;
    E: wo(b1). Scores of pair j+1 are emitted before AV of pair j.
  * PSUM: {pp,vp,fin,av}x3 + {rp}x1 + {sc [128,2,512]}x2x2banks = 8 banks.
"""

import numpy as np
import ml_dtypes

BF16 = ml_dtypes.bfloat16

B, S, D = 16, 512, 1024
H, HD = 16, 64
NCORES = 8
BPC = B // NCORES          # batches per core
T = BPC * S                # tokens per core
KC = D // 128              # 8 contraction chunks

_CACHE = {}


def _rope_tables():
    inv_freq = 1.0 / (10000.0 ** (np.arange(0, HD, 2, dtype=np.float32) / HD))
    t = np.arange(S, dtype=np.float32)
    freqs = np.outer(t, inv_freq)                    # [S, 32]
    emb = np.concatenate([freqs, freqs], -1)         # [S, 64]
    return np.cos(emb), np.sin(emb)                  # [S, 64] fp32


# Head-dim permutation: with per-head feature order [0:16, 32:48, 16:32,
# 48:64], rotate_half becomes a 16-lane swap within each 32-partition
# quadrant — exactly what the DVE stream_shuffle reshape block can do.
# Scores are invariant since q and k share the permutation.
PERM64 = np.array(list(range(0, 16)) + list(range(32, 48))
                  + list(range(16, 32)) + list(range(48, 64)))
SHUF_MASK = [(i + 16) % 32 for i in range(32)]


def _host_consts():
    cos, sin = _rope_tables()
    cols = np.arange(T) % S
    cosP = cos[:, PERM64]                            # [S, 64] permuted dims
    sinP = sin[:, PERM64]
    sign = np.where((np.arange(64) % 32) < 16, -1.0, 1.0)
    sinP = sinP * sign
    cosT = np.ascontiguousarray(np.tile(cosP[cols].T, (2, 1))).astype(BF16)  # [128, T]
    sinT = np.ascontiguousarray(np.tile(sinP[cols].T, (2, 1))).astype(BF16)
    mask01 = (np.arange(128)[None, :] >= np.arange(128)[:, None]).astype(BF16)  # [kt, qt]
    return cosT, sinT, mask01


def _permute_qk_cols(WT):
    """WT [in, out]: permute out-feature columns by PERM64 within each head."""
    colidx = (np.arange(D) // HD) * HD
    colidx = colidx + PERM64[np.arange(D) % HD]
    return np.ascontiguousarray(WT[:, colidx])


def _build_bass():
    import concourse.bacc as bacc
    import concourse.tile as tile
    import concourse.mybir as mybir

    dt = mybir.dt
    f32, bf16 = dt.float32, dt.bfloat16
    Exp = mybir.ActivationFunctionType.Exp

    nc = bacc.Bacc("TRN2", target_bir_lowering=False, debug=False, enable_asserts=False)

    xT_d = nc.dram_tensor("xT", [D, T], bf16, kind="ExternalInput").ap()
    wq_d = nc.dram_tensor("WqT", [D, D], bf16, kind="ExternalInput").ap()
    wk_d = nc.dram_tensor("WkT", [D, D], bf16, kind="ExternalInput").ap()
    wv_d = nc.dram_tensor("WvT", [D, D], bf16, kind="ExternalInput").ap()
    wo_d = nc.dram_tensor("WoT", [D, D], bf16, kind="ExternalInput").ap()
    cos_d = nc.dram_tensor("cosT", [128, T], bf16, kind="ExternalInput").ap()
    sin_d = nc.dram_tensor("sinT", [128, T], bf16, kind="ExternalInput").ap()
    mask_d = nc.dram_tensor("mask01", [128, 128], bf16, kind="ExternalInput").ap()
    out_d = nc.dram_tensor("outT", [D, T], bf16, kind="ExternalOutput").ap()

    with tile.TileContext(nc) as tc:
        with (
            tc.tile_pool(name="consts", bufs=1) as consts,
            tc.tile_pool(name="persist", bufs=1) as persist,
            tc.tile_pool(name="work", bufs=2) as work,
            tc.tile_pool(name="expp", bufs=2) as expp,
            tc.tile_pool(name="psum", bufs=1, space="PSUM") as ps,
        ):
            # ---------------- HAM warmup operands: memset before any DMA
            # kicks on the gpsimd queue so junk matmuls run from ~6us
            junk_w = consts.tile([128, 128], bf16, name="junk_w")
            junk_x = consts.tile([128, 256], bf16, name="junk_x")
            nc.gpsimd.memset(junk_w, 0.0)
            nc.gpsimd.memset(junk_x, 0.0)

            # ---------------- input loads, first-use order
            def load(dram, rows, shape, nm, eng):
                t_ = consts.tile(shape, bf16, name=nm)
                eng.dma_start(out=t_, in_=dram[rows[0]:rows[1], :] if rows else dram)
                return t_

            def kr(k):
                return (k * 128, (k + 1) * 128)

            # interleave x/wq kicks across both hwdge queues
            xT, wq = [None] * KC, [None] * KC
            for k in range(KC):
                xT[k] = load(xT_d, kr(k), [128, T], f"xT{k}",
                             nc.sync if k % 2 == 0 else nc.scalar)
                wq[k] = load(wq_d, kr(k), [128, D], f"wq{k}",
                             nc.scalar if k % 2 == 0 else nc.sync)
            cosT = load(cos_d, None, [128, T], "cosT", nc.gpsimd)
            sinT = load(sin_d, None, [128, T], "sinT", nc.gpsimd)
            wk = [load(wk_d, kr(k), [128, D], f"wk{k}",
                       nc.sync if k % 2 == 0 else nc.scalar) for k in range(KC)]
            wv = [load(wv_d, kr(k), [128, D], f"wv{k}",
                       nc.scalar if k % 2 == 0 else nc.sync) for k in range(KC)]
            mask = load(mask_d, None, [128, 128], "mask", nc.gpsimd)
            wo = [load(wo_d, kr(k), [128, D], f"wo{k}",
                       nc.sync if k % 2 == 0 else nc.scalar) for k in range(KC)]
            mask2 = mask.unsqueeze(1).broadcast_to([128, 2, 128])

            # junk matmuls: sustained PE activity during the DMA-gated ramp
            def emit_warm(n):
                warm_ps = ps.tile([128, 256], f32, name="warm", tag="sc", bufs=2)
                for _ in range(n):
                    nc.tensor.matmul(warm_ps, junk_w, junk_x, start=True, stop=True)

            emit_warm(18)

            # ---------------- persistent intermediates
            qrot = [persist.tile([128, T], bf16, name=f"qrot{m}") for m in range(KC)]
            krot = [persist.tile([128, T], bf16, name=f"krot{m}") for m in range(KC)]
            vsb = [[persist.tile([128, H * 65], bf16, name=f"vsb{b}_{t_}")
                    for t_ in range(4)] for b in range(BPC)]
            att = [persist.tile([128, T], bf16, name=f"att{m}") for m in range(KC)]

            for b in range(BPC):
                for t_ in range(4):
                    vt = vsb[b][t_].rearrange("p (h w) -> p h w", w=65)
                    nc.gpsimd.memset(vt[:, :, 64:65], 1.0)

            # ---------------- emitters
            def emit_qk_group(b, w_sb, rot, m, pre_act=True):
                cols = slice(b * S, (b + 1) * S)
                pp = ps.tile([128, S], f32, name="pp", tag="mm1", bufs=4)
                for k in range(KC):
                    nc.tensor.matmul(
                        pp, w_sb[k][:, m * 128:(m + 1) * 128], xT[k][:, cols],
                        start=(k == 0), stop=(k == KC - 1))
                pre = work.tile([128, S], bf16, name="pre", tag="pre", bufs=2)
                if pre_act:
                    nc.scalar.copy(pre, pp)          # ACT psum -> sbuf bf16
                else:
                    nc.vector.tensor_copy(pre, pp)
                # rotate_half == 16-lane swap per 32-quadrant in PERM64 layout
                shf = work.tile([128, S], bf16, name="shf", tag="shf", bufs=2)
                nc.vector.stream_shuffle(shf, pre, SHUF_MASK)
                t1 = work.tile([128, S], bf16, name="t1", tag="t1", bufs=2)
                nc.vector.tensor_mul(t1, pre, cosT[:, cols])
                t2 = work.tile([128, S], bf16, name="t2", tag="t2", bufs=2)
                nc.vector.tensor_mul(t2, shf, sinT[:, cols])
                nc.vector.tensor_add(rot[m][:, cols], t1, t2)

            def emit_v_group(b, tch, nh, v_act=False):
                vt = vsb[b][tch].rearrange("p (h w) -> p h w", w=65)
                vp = ps.tile([128, S], f32, name="vp", tag="mm1", bufs=4)
                for k in range(KC):
                    nc.tensor.matmul(
                        vp, xT[k][:, b * S + tch * 128: b * S + (tch + 1) * 128],
                        wv[k][:, nh * S:(nh + 1) * S],
                        start=(k == 0), stop=(k == KC - 1))
                dst = vt[:, nh * 8:(nh + 1) * 8, 0:64]
                src = vp.rearrange("p (h w) -> p h w", w=64)
                if v_act:
                    nc.scalar.copy(dst, src)
                else:
                    nc.vector.tensor_copy(dst, src)

            def emit_attn_sc(b, j):
                """scores + exp + mask for head pair (2j, 2j+1); returns ex tiles."""
                mh = j
                exs = []
                for i in range(4):
                    lo = i * 128
                    sc = ps.tile([128, 2, S], f32, name="sc", tag="sc", bufs=2)
                    for hi, p0 in ((0, 0), (1, 64)):
                        nc.tensor.matmul(
                            sc[:, hi, 0:S - lo],
                            krot[mh][p0:p0 + 64, b * S + lo: b * S + lo + 128],
                            qrot[mh][p0:p0 + 64, b * S + lo: (b + 1) * S],
                            start=True, stop=True)
                    ex = expp.tile([128, 2, S], bf16, name="ex", tag=f"ex{i}", bufs=2)
                    nc.scalar.activation(ex[:, :, lo:S], sc[:, :, 0:S - lo], Exp, scale=0.125)
                    nc.vector.tensor_mul(ex[:, :, lo:lo + 128], ex[:, :, lo:lo + 128], mask2)
                    exs.append(ex)
                return exs

            def emit_attn_av(b, j, exs):
                """AV + softmax-normalize for both heads of pair j."""
                mh = j
                bcols = slice(b * S, (b + 1) * S)
                for h in (2 * j, 2 * j + 1):
                    hi, p0 = h % 2, (h % 2) * 64
                    av = ps.tile([128, S], f32, name="av", tag="mm1", bufs=4)
                    for i in range(4):
                        lo = i * 128
                        nc.tensor.matmul(
                            av[0:65, lo:S],
                            vsb[b][i][:, h * 65: h * 65 + 65],
                            exs[i][:, hi, lo:S],
                            start=(i == 0), stop=(i == 3), skip_group_check=True)
                    # free the psum bank quickly (ACT), then the reciprocal
                    # round-trip works from SBUF without holding PSUM
                    avs = work.tile([65, S], bf16, name="avs", tag="avs", bufs=4)
                    nc.scalar.copy(avs, av[0:65, :])
                    st = work.tile([128, 4], bf16, name="st", tag="st", bufs=3)
                    nc.sync.dma_start(out=st, in_=avs[64:65, :])
                    rt_ = work.tile([128, 4], bf16, name="rt", tag="rt", bufs=3)
                    with nc.allow_low_precision(reason="softmax 1/sum in bf16"):
                        nc.vector.reciprocal(rt_, st)
                    rr = work.tile([1, S], bf16, name="rr", tag="rr", bufs=3)
                    nc.sync.dma_start(out=rr, in_=rt_)
                    rb = work.tile([64, S], bf16, name="rb", tag="rb", bufs=3)
                    nc.gpsimd.partition_broadcast(rb, rr)
                    nc.vector.tensor_mul(att[mh][p0:p0 + 64, bcols], avs[0:64, :], rb)

            def emit_wo_group(b, m, ob_act=False):
                bcols = slice(b * S, (b + 1) * S)
                fin = ps.tile([128, S], f32, name="fin", tag="mm1", bufs=4)
                for k in range(KC):
                    nc.tensor.matmul(
                        fin, wo[k][:, m * 128:(m + 1) * 128], att[k][:, bcols],
                        start=(k == 0), stop=(k == KC - 1))
                ob = work.tile([128, S], bf16, name="ob", tag="ob", bufs=2)
                if ob_act:
                    nc.scalar.copy(ob, fin)
                else:
                    nc.vector.tensor_copy(ob, fin)
                eng = nc.scalar if (b * KC + m) % 2 == 0 else nc.sync
                eng.dma_start(out=out_d[m * 128:(m + 1) * 128, bcols], in_=ob)

            # ---------------- schedule
            # A: b0 projections, all q first (wk still streaming in), then k
            for m in range(KC):
                if m < 6:
                    emit_warm(3)
                emit_qk_group(0, wq, qrot, m)
            for m in range(KC):
                emit_qk_group(0, wk, krot, m)
            # B: v(b0, nh0) — needed by first attn pairs
            for tch in range(4):
                emit_v_group(0, tch, 0, v_act=True)
            # C: attn b0 pipelined; fillers: qk(b1), v(b0,nh1), v(b1,nh0)
            exs_c = emit_attn_sc(0, 0)
            for j in range(8):
                if j % 2 == 0:
                    emit_qk_group(1, wq, qrot, j // 2, pre_act=False)
                else:
                    emit_qk_group(1, wk, krot, j // 2, pre_act=False)
                nxt = emit_attn_sc(0, j + 1) if j < 7 else None
                if j < 4:
                    emit_v_group(0, j, 1, v_act=False)
                else:
                    emit_v_group(1, j - 4, 0, v_act=False)
                emit_attn_av(0, j, exs_c)
                exs_c = nxt
            # D: attn b1 pipelined; fillers: qk(b1) tail, v(b1,nh1), wo(b0)
            fillers = [
                [("qk", 4)], [("qk", 5)], [("v", 0), ("v", 1)],
                [("v", 2), ("v", 3)], [("qk", 6), ("wo", 0)],
                [("qk", 7), ("wo", 1)], [("wo", 2), ("wo", 3), ("wo", 4)],
                [("wo", 5), ("wo", 6), ("wo", 7)],
            ]
            exs_c = emit_attn_sc(1, 0)
            for j in range(8):
                fl = list(fillers[j])
                if fl:
                    kind, a = fl.pop(0)
                    if kind == "qk":
                        emit_qk_group(1, wq, qrot, a, pre_act=False)
                        emit_qk_group(1, wk, krot, a, pre_act=False)
                    elif kind == "v":
                        emit_v_group(1, a, 1, v_act=False)
                    else:
                        emit_wo_group(0, a, ob_act=False)
                nxt = emit_attn_sc(1, j + 1) if j < 7 else None
                for kind, a in fl:
                    if kind == "qk":
                        emit_qk_group(1, wq, qrot, a, pre_act=False)
                        emit_qk_group(1, wk, krot, a, pre_act=False)
                    elif kind == "v":
                        emit_v_group(1, a, 1, v_act=False)
                    else:
                        emit_wo_group(0, a, ob_act=False)
                emit_attn_av(1, j, exs_c)
                exs_c = nxt
            # E: wo b1 (DVE and ACT both free here; alternate copies)
            for m in range(KC):
                emit_wo_group(1, m, ob_act=(m % 2 == 0))

    nc.compile()
    return nc


def _get_nc():
    if "nc" not in _CACHE:
        _CACHE["nc"] = _build_bass()
    return _CACHE["nc"]


def make_in_maps(x, Wq, Wk, Wv, Wo):
    """Host-side shard + layout prep: one input dict per core."""
    cosT, sinT, mask01 = _host_consts()
    shared = {
        "WqT": _permute_qk_cols(Wq.T).astype(BF16),
        "WkT": _permute_qk_cols(Wk.T).astype(BF16),
        "WvT": np.ascontiguousarray(Wv.T).astype(BF16),
        "WoT": np.ascontiguousarray(Wo.T).astype(BF16),
        "cosT": cosT,
        "sinT": sinT,
        "mask01": mask01,
    }
    in_maps = []
    for c in range(NCORES):
        xc = x[c * BPC:(c + 1) * BPC]  # [BPC, S, D]
        xT = np.ascontiguousarray(xc.transpose(2, 0, 1).reshape(D, T)).astype(BF16)
        in_maps.append({"xT": xT, **shared})
    return in_maps


def assemble(results):
    """results: list (per core) of {"outT": [D, T] bf16} -> [B, S, D] fp32."""
    out = np.empty((B, S, D), np.float32)
    for c in range(NCORES):
        oT = np.asarray(results[c]["outT"]).astype(np.float32)
        out[c * BPC:(c + 1) * BPC] = oT.reshape(D, BPC, S).transpose(1, 2, 0)
    return out


def run(x, Wq, Wk, Wv, Wo, trace=False, **run_kwargs):
    from concourse.bass_utils import run_bass_kernel_spmd
    nc = _get_nc()
    in_maps = make_in_maps(x, Wq, Wk, Wv, Wo)
    res = run_bass_kernel_spmd(
        nc, in_maps, core_ids=list(range(NCORES)), trace=trace, **run_kwargs)
    return assemble(res.results), res


def kernel(x, Wq, Wk, Wv, Wo):
    out, _ = run(np.asarray(x), np.asarray(Wq), np.asarray(Wk),
                 np.asarray(Wv), np.asarray(Wo))
    return out


if __name__ == "__main__":
    rng = np.random.default_rng(0)
    scale = 1.0 / np.sqrt(D)
    inputs = {
        "x": rng.standard_normal((B, S, D), dtype=np.float32),
        "Wq": (rng.standard_normal((D, D), dtype=np.float32) * scale),
        "Wk": (rng.standard_normal((D, D), dtype=np.float32) * scale),
        "Wv": (rng.standard_normal((D, D), dtype=np.float32) * scale),
        "Wo": (rng.standard_normal((D, D), dtype=np.float32) * scale),
    }
    out = kernel(**inputs)
    print("out", out.shape, out.dtype, float(np.abs(out).max()))


# revision 19
# speedup vs baseline: 1.0635x; 1.0635x over previous
"""Multi-head attention (RoPE + causal softmax) Trainium2 Bass kernel.

Problem: nn_MultiHeadAttention (B=16, S=512, D=1024, H=16, Hd=64).
Sharding: data-parallel over batch - 2 batches per core on 8 NeuronCores.

v3. Key structure (per core, T=1024 tokens feature-major):
  * Inputs stream in first-use order on the Sync HWDGE queue (+ Scalar for
    the early critical tiles, gpsimd for consts) so the PE starts ~2us in.
  * Softmax normalize: ACT copies av psum->sbuf bf16 (frees PSUM in
    ~1.8us), sums row [1,512] -> [128,4] via Sync HWDGE sbuf-sbuf DMA,
    DVE reciprocal, DMA back, gpsimd partition-broadcast, bf16 2x mul.
  * RoPE: pre (ACT copy), rpre (DVE cast), then 3 bf16 2x DVE TTs.
  * Causal mask muls batched over both heads via stride-0 broadcast AP.
  * bf16 output, DVE (phase D) / ACT (phase E) psum->bf16 copies,
    out DMA on gpsimd queue.
  * Schedule: A: all b0 q/k groups; B: v(b0,nh0); C: attn(b0) pairs with
    qk(b1)/v fillers; D: attn(b1) pairs with qk-tail/v/wo(b0) fillers;
    E: wo(b1). Scores of pair j+1 are emitted before AV of pair j.
  * PSUM: {pp,vp,fin,av}x3 + {rp}x1 + {sc [128,2,512]}x2x2banks = 8 banks.
"""

import numpy as np
import ml_dtypes

BF16 = ml_dtypes.bfloat16

B, S, D = 16, 512, 1024
H, HD = 16, 64
NCORES = 8
BPC = B // NCORES          # batches per core
T = BPC * S                # tokens per core
KC = D // 128              # 8 contraction chunks

_CACHE = {}


def _rope_tables():
    inv_freq = 1.0 / (10000.0 ** (np.arange(0, HD, 2, dtype=np.float32) / HD))
    t = np.arange(S, dtype=np.float32)
    freqs = np.outer(t, inv_freq)                    # [S, 32]
    emb = np.concatenate([freqs, freqs], -1)         # [S, 64]
    return np.cos(emb), np.sin(emb)                  # [S, 64] fp32


# Head-dim permutation: with per-head feature order [0:16, 32:48, 16:32,
# 48:64], rotate_half becomes a 16-lane swap within each 32-partition
# quadrant — exactly what the DVE stream_shuffle reshape block can do.
# Scores are invariant since q and k share the permutation.
PERM64 = np.array(list(range(0, 16)) + list(range(32, 48))
                  + list(range(16, 32)) + list(range(48, 64)))
SHUF_MASK = [(i + 16) % 32 for i in range(32)]


def _host_consts():
    cos, sin = _rope_tables()
    cols = np.arange(T) % S
    cosP = cos[:, PERM64]                            # [S, 64] permuted dims
    sinP = sin[:, PERM64]
    sign = np.where((np.arange(64) % 32) < 16, -1.0, 1.0)
    sinP = sinP * sign
    cosT = np.ascontiguousarray(np.tile(cosP[cols].T, (2, 1))).astype(BF16)  # [128, T]
    sinT = np.ascontiguousarray(np.tile(sinP[cols].T, (2, 1))).astype(BF16)
    mask01 = (np.arange(128)[None, :] >= np.arange(128)[:, None]).astype(BF16)  # [kt, qt]
    return cosT, sinT, mask01


def _permute_qk_cols(WT):
    """WT [in, out]: permute out-feature columns by PERM64 within each head."""
    colidx = (np.arange(D) // HD) * HD
    colidx = colidx + PERM64[np.arange(D) % HD]
    return np.ascontiguousarray(WT[:, colidx])


def _build_bass():
    import concourse.bacc as bacc
    import concourse.tile as tile
    import concourse.mybir as mybir

    dt = mybir.dt
    f32, bf16 = dt.float32, dt.bfloat16
    Exp = mybir.ActivationFunctionType.Exp

    nc = bacc.Bacc("TRN2", target_bir_lowering=False, debug=False, enable_asserts=False)

    xT_d = nc.dram_tensor("xT", [D, T], bf16, kind="ExternalInput").ap()
    wq_d = nc.dram_tensor("WqT", [D, D], bf16, kind="ExternalInput").ap()
    wk_d = nc.dram_tensor("WkT", [D, D], bf16, kind="ExternalInput").ap()
    wv_d = nc.dram_tensor("WvT", [D, D], bf16, kind="ExternalInput").ap()
    wo_d = nc.dram_tensor("WoT", [D, D], bf16, kind="ExternalInput").ap()
    cos_d = nc.dram_tensor("cosT", [128, T], bf16, kind="ExternalInput").ap()
    sin_d = nc.dram_tensor("sinT", [128, T], bf16, kind="ExternalInput").ap()
    mask_d = nc.dram_tensor("mask01", [128, 128], bf16, kind="ExternalInput").ap()
    out_d = nc.dram_tensor("outT", [D, T], bf16, kind="ExternalOutput").ap()

    with tile.TileContext(nc) as tc:
        with (
            tc.tile_pool(name="consts", bufs=1) as consts,
            tc.tile_pool(name="persist", bufs=1) as persist,
            tc.tile_pool(name="work", bufs=2) as work,
            tc.tile_pool(name="expp", bufs=2) as expp,
            tc.tile_pool(name="psum", bufs=1, space="PSUM") as ps,
        ):
            # ---------------- HAM warmup operands: memset before any DMA
            # kicks on the gpsimd queue so junk matmuls run from ~6us
            junk_w = consts.tile([128, 128], bf16, name="junk_w")
            junk_x = consts.tile([128, 256], bf16, name="junk_x")
            nc.gpsimd.memset(junk_w, 0.0)
            nc.gpsimd.memset(junk_x, 0.0)

            # ---------------- input loads, first-use order
            def load(dram, rows, shape, nm, eng):
                t_ = consts.tile(shape, bf16, name=nm)
                eng.dma_start(out=t_, in_=dram[rows[0]:rows[1], :] if rows else dram)
                return t_

            def kr(k):
                return (k * 128, (k + 1) * 128)

            # interleave x/wq kicks across both hwdge queues
            xT, wq = [None] * KC, [None] * KC
            for k in range(KC):
                xT[k] = load(xT_d, kr(k), [128, T], f"xT{k}",
                             nc.sync if k % 2 == 0 else nc.scalar)
                wq[k] = load(wq_d, kr(k), [128, D], f"wq{k}",
                             nc.scalar if k % 2 == 0 else nc.sync)
            cosT = load(cos_d, None, [128, T], "cosT", nc.gpsimd)
            sinT = load(sin_d, None, [128, T], "sinT", nc.gpsimd)
            wk = [load(wk_d, kr(k), [128, D], f"wk{k}", nc.sync)
                  for k in range(KC)]
            wv = [load(wv_d, kr(k), [128, D], f"wv{k}", nc.sync)
                  for k in range(KC)]
            mask = load(mask_d, None, [128, 128], "mask", nc.gpsimd)
            wo = [load(wo_d, kr(k), [128, D], f"wo{k}", nc.sync)
                  for k in range(KC)]
            mask2 = mask.unsqueeze(1).broadcast_to([128, 2, 128])

            # junk matmuls: sustained PE activity during the DMA-gated ramp
            def emit_warm(n):
                warm_ps = ps.tile([128, 256], f32, name="warm", tag="sc", bufs=2)
                for _ in range(n):
                    nc.tensor.matmul(warm_ps, junk_w, junk_x, start=True, stop=True)

            emit_warm(18)

            # ---------------- persistent intermediates
            qrot = [persist.tile([128, T], bf16, name=f"qrot{m}") for m in range(KC)]
            krot = [persist.tile([128, T], bf16, name=f"krot{m}") for m in range(KC)]
            vsb = [[persist.tile([128, H * 65], bf16, name=f"vsb{b}_{t_}")
                    for t_ in range(4)] for b in range(BPC)]
            att = [persist.tile([128, T], bf16, name=f"att{m}") for m in range(KC)]

            for b in range(BPC):
                for t_ in range(4):
                    vt = vsb[b][t_].rearrange("p (h w) -> p h w", w=65)
                    nc.gpsimd.memset(vt[:, :, 64:65], 1.0)

            # ---------------- emitters
            def emit_qk_group(b, w_sb, rot, m, pre_act=True):
                cols = slice(b * S, (b + 1) * S)
                pp = ps.tile([128, S], f32, name="pp", tag="mm1", bufs=4)
                for k in range(KC):
                    nc.tensor.matmul(
                        pp, w_sb[k][:, m * 128:(m + 1) * 128], xT[k][:, cols],
                        start=(k == 0), stop=(k == KC - 1))
                pre = work.tile([128, S], bf16, name="pre", tag="pre", bufs=2)
                if pre_act:
                    nc.scalar.copy(pre, pp)          # ACT psum -> sbuf bf16
                else:
                    nc.vector.tensor_copy(pre, pp)
                # rotate_half == 16-lane swap per 32-quadrant in PERM64 layout
                shf = work.tile([128, S], bf16, name="shf", tag="shf", bufs=2)
                nc.vector.stream_shuffle(shf, pre, SHUF_MASK)
                t1 = work.tile([128, S], bf16, name="t1", tag="t1", bufs=2)
                nc.vector.tensor_mul(t1, pre, cosT[:, cols])
                t2 = work.tile([128, S], bf16, name="t2", tag="t2", bufs=2)
                nc.vector.tensor_mul(t2, shf, sinT[:, cols])
                nc.vector.tensor_add(rot[m][:, cols], t1, t2)

            def emit_v_group(b, tch, nh, v_act=False):
                vt = vsb[b][tch].rearrange("p (h w) -> p h w", w=65)
                vp = ps.tile([128, S], f32, name="vp", tag="mm1", bufs=4)
                for k in range(KC):
                    nc.tensor.matmul(
                        vp, xT[k][:, b * S + tch * 128: b * S + (tch + 1) * 128],
                        wv[k][:, nh * S:(nh + 1) * S],
                        start=(k == 0), stop=(k == KC - 1))
                dst = vt[:, nh * 8:(nh + 1) * 8, 0:64]
                src = vp.rearrange("p (h w) -> p h w", w=64)
                if v_act:
                    nc.scalar.copy(dst, src)
                else:
                    nc.vector.tensor_copy(dst, src)

            def emit_attn_sc(b, j):
                """scores + exp + mask for head pair (2j, 2j+1); returns ex tiles."""
                mh = j
                exs = []
                for i in range(4):
                    lo = i * 128
                    sc = ps.tile([128, 2, S], f32, name="sc", tag="sc", bufs=2)
                    for hi, p0 in ((0, 0), (1, 64)):
                        nc.tensor.matmul(
                            sc[:, hi, 0:S - lo],
                            krot[mh][p0:p0 + 64, b * S + lo: b * S + lo + 128],
                            qrot[mh][p0:p0 + 64, b * S + lo: (b + 1) * S],
                            start=True, stop=True)
                    ex = expp.tile([128, 2, S], bf16, name="ex", tag=f"ex{i}", bufs=2)
                    nc.scalar.activation(ex[:, :, lo:S], sc[:, :, 0:S - lo], Exp, scale=0.125)
                    nc.vector.tensor_mul(ex[:, :, lo:lo + 128], ex[:, :, lo:lo + 128], mask2)
                    exs.append(ex)
                return exs

            def emit_attn_av(b, j, exs):
                """AV + softmax-normalize for both heads of pair j."""
                mh = j
                bcols = slice(b * S, (b + 1) * S)
                for h in (2 * j, 2 * j + 1):
                    hi, p0 = h % 2, (h % 2) * 64
                    av = ps.tile([128, S], f32, name="av", tag="mm1", bufs=4)
                    for i in range(4):
                        lo = i * 128
                        nc.tensor.matmul(
                            av[0:65, lo:S],
                            vsb[b][i][:, h * 65: h * 65 + 65],
                            exs[i][:, hi, lo:S],
                            start=(i == 0), stop=(i == 3), skip_group_check=True)
                    # free the psum bank quickly (ACT), then the reciprocal
                    # round-trip works from SBUF without holding PSUM
                    avs = work.tile([65, S], bf16, name="avs", tag="avs", bufs=4)
                    nc.scalar.copy(avs, av[0:65, :])
                    st = work.tile([128, 4], bf16, name="st", tag="st", bufs=3)
                    nc.sync.dma_start(out=st, in_=avs[64:65, :])
                    rt_ = work.tile([128, 4], bf16, name="rt", tag="rt", bufs=3)
                    with nc.allow_low_precision(reason="softmax 1/sum in bf16"):
                        nc.vector.reciprocal(rt_, st)
                    rr = work.tile([1, S], bf16, name="rr", tag="rr", bufs=3)
                    nc.sync.dma_start(out=rr, in_=rt_)
                    rb = work.tile([64, S], bf16, name="rb", tag="rb", bufs=3)
                    nc.gpsimd.partition_broadcast(rb, rr)
                    nc.vector.tensor_mul(att[mh][p0:p0 + 64, bcols], avs[0:64, :], rb)

            def emit_wo_group(b, m, ob_act=False):
                bcols = slice(b * S, (b + 1) * S)
                fin = ps.tile([128, S], f32, name="fin", tag="mm1", bufs=4)
                for k in range(KC):
                    nc.tensor.matmul(
                        fin, wo[k][:, m * 128:(m + 1) * 128], att[k][:, bcols],
                        start=(k == 0), stop=(k == KC - 1))
                ob = work.tile([128, S], bf16, name="ob", tag="ob", bufs=2)
                if ob_act:
                    nc.scalar.copy(ob, fin)
                else:
                    nc.vector.tensor_copy(ob, fin)
                eng = nc.scalar if (b * KC + m) % 2 == 0 else nc.sync
                eng.dma_start(out=out_d[m * 128:(m + 1) * 128, bcols], in_=ob)

            # ---------------- schedule
            # A: b0 projections, all q first (wk still streaming in), then k
            for m in range(KC):
                if m < 6:
                    emit_warm(3)
                emit_qk_group(0, wq, qrot, m)
            for m in range(KC):
                emit_qk_group(0, wk, krot, m)
            # B: v(b0, nh0) — needed by first attn pairs
            for tch in range(4):
                emit_v_group(0, tch, 0, v_act=True)
            # C: attn b0 pipelined; fillers: qk(b1), v(b0,nh1), v(b1,nh0)
            exs_c = emit_attn_sc(0, 0)
            for j in range(8):
                if j % 2 == 0:
                    emit_qk_group(1, wq, qrot, j // 2, pre_act=False)
                else:
                    emit_qk_group(1, wk, krot, j // 2, pre_act=False)
                nxt = emit_attn_sc(0, j + 1) if j < 7 else None
                if j < 4:
                    emit_v_group(0, j, 1, v_act=False)
                else:
                    emit_v_group(1, j - 4, 0, v_act=False)
                emit_attn_av(0, j, exs_c)
                exs_c = nxt
            # D: attn b1 pipelined; fillers: qk(b1) tail, v(b1,nh1), wo(b0)
            fillers = [
                [("qk", 4)], [("qk", 5)], [("v", 0), ("v", 1)],
                [("v", 2), ("v", 3)], [("qk", 6), ("wo", 0)],
                [("qk", 7), ("wo", 1)], [("wo", 2), ("wo", 3), ("wo", 4)],
                [("wo", 5), ("wo", 6), ("wo", 7)],
            ]
            exs_c = emit_attn_sc(1, 0)
            for j in range(8):
                fl = list(fillers[j])
                if fl:
                    kind, a = fl.pop(0)
                    if kind == "qk":
                        emit_qk_group(1, wq, qrot, a)
                        emit_qk_group(1, wk, krot, a)
                    elif kind == "v":
                        emit_v_group(1, a, 1, v_act=False)
                    else:
                        emit_wo_group(0, a, ob_act=False)
                nxt = emit_attn_sc(1, j + 1) if j < 7 else None
                for kind, a in fl:
                    if kind == "qk":
                        emit_qk_group(1, wq, qrot, a)
                        emit_qk_group(1, wk, krot, a)
                    elif kind == "v":
                        emit_v_group(1, a, 1, v_act=False)
                    else:
                        emit_wo_group(0, a, ob_act=False)
                emit_attn_av(1, j, exs_c)
                exs_c = nxt
            # E: wo b1 (DVE and ACT both free here; alternate copies)
            for m in range(KC):
                emit_wo_group(1, m, ob_act=(m % 2 == 0))

    nc.compile()
    return nc


def _get_nc():
    if "nc" not in _CACHE:
        _CACHE["nc"] = _build_bass()
    return _CACHE["nc"]


def make_in_maps(x, Wq, Wk, Wv, Wo):
    """Host-side shard + layout prep: one input dict per core."""
    cosT, sinT, mask01 = _host_consts()
    shared = {
        "WqT": _permute_qk_cols(Wq.T).astype(BF16),
        "WkT": _permute_qk_cols(Wk.T).astype(BF16),
        "WvT": np.ascontiguousarray(Wv.T).astype(BF16),
        "WoT": np.ascontiguousarray(Wo.T).astype(BF16),
        "cosT": cosT,
        "sinT": sinT,
        "mask01": mask01,
    }
    in_maps = []
    for c in range(NCORES):
        xc = x[c * BPC:(c + 1) * BPC]  # [BPC, S, D]
        xT = np.ascontiguousarray(xc.transpose(2, 0, 1).reshape(D, T)).astype(BF16)
        in_maps.append({"xT": xT, **shared})
    return in_maps


def assemble(results):
    """results: list (per core) of {"outT": [D, T] bf16} -> [B, S, D] fp32."""
    out = np.empty((B, S, D), np.float32)
    for c in range(NCORES):
        oT = np.asarray(results[c]["outT"]).astype(np.float32)
        out[c * BPC:(c + 1) * BPC] = oT.reshape(D, BPC, S).transpose(1, 2, 0)
    return out


def run(x, Wq, Wk, Wv, Wo, trace=False, **run_kwargs):
    from concourse.bass_utils import run_bass_kernel_spmd
    nc = _get_nc()
    in_maps = make_in_maps(x, Wq, Wk, Wv, Wo)
    res = run_bass_kernel_spmd(
        nc, in_maps, core_ids=list(range(NCORES)), trace=trace, **run_kwargs)
    return assemble(res.results), res


def kernel(x, Wq, Wk, Wv, Wo):
    out, _ = run(np.asarray(x), np.asarray(Wq), np.asarray(Wk),
                 np.asarray(Wv), np.asarray(Wo))
    return out


if __name__ == "__main__":
    rng = np.random.default_rng(0)
    scale = 1.0 / np.sqrt(D)
    inputs = {
        "x": rng.standard_normal((B, S, D), dtype=np.float32),
        "Wq": (rng.standard_normal((D, D), dtype=np.float32) * scale),
        "Wk": (rng.standard_normal((D, D), dtype=np.float32) * scale),
        "Wv": (rng.standard_normal((D, D), dtype=np.float32) * scale),
        "Wo": (rng.standard_normal((D, D), dtype=np.float32) * scale),
    }
    out = kernel(**inputs)
    print("out", out.shape, out.dtype, float(np.abs(out).max()))
